# revision 1
# baseline (speedup 1.0000x reference)
"""Trainium2 Bass kernel for nn_CoattentionModel (co-attention + conv-fusion + convGRU).

Sharding: token axis (3600 tokens = 60x60 image) padded to 64 rows (3840 tokens),
split 8 ways -> each core owns 8 image rows (480 tokens). Attention is computed
as A'[j,i] tiles (query-token j on partitions), softmax without max-subtraction
(logits verified <= ~40), attention output accumulated over 29 j-tiles in PSUM.
Softmax sum + gate row come from a 2-row matmul against [ones | g] per j-tile.
Normalize * sigmoid-gate * pad-valid mask fold into one per-column scale vector.
Matmuls run in float32r (full PE rate, ~1e-3 max rel err); the 3x3 conv path
runs in bf16 to fit SBUF. Per round: 2 edge AllGathers provide conv halos
(read back at rank-dynamic register offsets), 3 feature AllGathers rebuild the
full features for the next round's attention.
"""
import sys
for _p in ("/opt/trn_rl_repo", "/root/.axon_site/_ro/trn_rl_repo"):
    if _p not in sys.path:
        sys.path.insert(0, _p)

import numpy as np
import ml_dtypes

import concourse.bass as bass
import concourse.mybir as mybir
import concourse.tile as tile
from concourse import bacc
from concourse.bass_utils import run_bass_kernel_spmd
from concourse.masks import make_identity

F32 = mybir.dt.float32
F32R = mybir.dt.float32r
BF16 = mybir.dt.bfloat16
I32 = mybir.dt.int32
AF = mybir.ActivationFunctionType
MUL = mybir.AluOpType.mult

C = 256
HW = 60
D = HW * HW              # 3600
ROWS_PAD = 64
D_PAD = ROWS_PAD * HW    # 3840
NCORE = 8
SLAB = D_PAD // NCORE    # 480
PW = HW + 2              # padded image width
ROUNDS = 5
JT = [(s, min(s + 128, D)) for s in range(0, D, 128)]  # 29 j-tiles over REAL tokens
NJT = len(JT)

# attention list: (E feature, Q feature), grouped in pairs sharing Q
ATTS = [(0, 1), (2, 1), (0, 2), (1, 2), (1, 0), (2, 0)]
PAIRS = [(1, [0, 1]), (2, [2, 3]), (0, [4, 5])]  # (Q feature, att indices)
# conv d consumes (attA, attB) channel-concat; GRU prev = feature d
CONV_PARTS = [(0, 2), (4, 3), (5, 1)]
# edge AllGather membership: AG-a = atts {0, 2} (ready after pair2) -> conv1
#                            AG-b = atts {1, 3, 4, 5} -> conv2, conv3
AG_A_ATTS = [0, 2]
AG_B_ATTS = [1, 3, 4, 5]


def r32(ap):
    return ap.bitcast(F32R)


def _build_nc():
    nc = bacc.Bacc("TRN2", target_bir_lowering=False, debug=False,
                   num_devices=NCORE)

    # ---------------- I/O ----------------
    featQ = nc.dram_tensor("featQ", [3, 2, 128, D], F32, kind="ExternalInput")
    feat_slab = nc.dram_tensor("feat_slab", [3, 2, 128, SLAB], F32,
                               kind="ExternalInput")
    W_linT = nc.dram_tensor("W_linT", [2, 128, 256], F32, kind="ExternalInput")
    W_gate_r = nc.dram_tensor("W_gate_r", [2, 128, 4], F32, kind="ExternalInput")
    W_cfT = nc.dram_tensor("W_cfT", [9, 4, 128, 256], BF16, kind="ExternalInput")
    b_cf2 = nc.dram_tensor("b_cf2", [2, 128], F32, kind="ExternalInput")
    gru_W = nc.dram_tensor("gru_W", [3, 4, 128, 256], F32, kind="ExternalInput")
    gru_b = nc.dram_tensor("gru_b", [3, 2, 128], F32, kind="ExternalInput")
    halo_bases = nc.dram_tensor("halo_bases", [1, 4], I32, kind="ExternalInput")
    halo_mask = nc.dram_tensor("halo_mask", [128, 2], F32, kind="ExternalInput")
    slab_valid = nc.dram_tensor("slab_valid", [1, SLAB], F32,
                                kind="ExternalInput")
    out_slab = nc.dram_tensor("out_slab", [3, 2, 128, SLAB], F32,
                              kind="ExternalOutput")

    with tile.TileContext(nc) as tc:
        import contextlib
        ctx = contextlib.ExitStack()
        with ctx:
            cst = ctx.enter_context(tc.tile_pool(name="cst", bufs=1))
            qfp = ctx.enter_context(tc.tile_pool(name="qfp", bufs=1))
            qtp = ctx.enter_context(tc.tile_pool(name="qtp", bufs=1))
            sgp = ctx.enter_context(tc.tile_pool(name="sgp", bufs=1))
            eslp = ctx.enter_context(tc.tile_pool(name="eslp", bufs=2))
            crp = ctx.enter_context(tc.tile_pool(name="crp", bufs=2))
            epp = ctx.enter_context(tc.tile_pool(name="epp", bufs=4))
            attp = ctx.enter_context(tc.tile_pool(name="attp", bufs=8))
            vecp = ctx.enter_context(tc.tile_pool(name="vecp", bufs=6))
            scbp = ctx.enter_context(tc.tile_pool(name="scbp", bufs=2))
            padp = ctx.enter_context(tc.tile_pool(name="padp", bufs=1))
            asbp = ctx.enter_context(tc.tile_pool(name="asbp", bufs=2))
            prvp = ctx.enter_context(tc.tile_pool(name="prvp", bufs=2))
            grup = ctx.enter_context(tc.tile_pool(name="grup", bufs=3))
            hp = ctx.enter_context(tc.tile_pool(name="hp", bufs=2))
            ps = ctx.enter_context(tc.tile_pool(name="ps", bufs=1, space="PSUM"))
            dr = ctx.enter_context(tc.tile_pool(name="dr", bufs=1, space="DRAM"))

            # ------------- constants -------------
            wlin_sb = cst.tile([128, 2, 256], F32R)
            nc.sync.dma_start(out=wlin_sb, in_=W_linT[:].rearrange("k p e -> p k e").bitcast(F32R))
            wgate_sb = cst.tile([128, 2, 4], F32R)
            nc.sync.dma_start(out=wgate_sb, in_=W_gate_r[:].rearrange("k p n -> p k n").bitcast(F32R))
            wcf_sb = cst.tile([128, 9, 4, 256], BF16)
            nc.sync.dma_start(out=wcf_sb, in_=W_cfT[:].rearrange("t k p o -> p t k o"))
            bcf_sb = cst.tile([128, 2], F32)
            nc.sync.dma_start(out=bcf_sb, in_=b_cf2[:].rearrange("c p -> p c"))
            gruw_sb = cst.tile([128, 3, 4, 256], F32R)
            nc.sync.dma_start(out=gruw_sb, in_=gru_W[:].rearrange("g k p o -> p g k o").bitcast(F32R))
            grub_sb = cst.tile([128, 3, 2], F32)
            nc.sync.dma_start(out=grub_sb, in_=gru_b[:].rearrange("g c p -> p g c"))
            hmask_sb = cst.tile([128, 2], F32)
            nc.sync.dma_start(out=hmask_sb, in_=halo_mask[:])
            valid_sb = cst.tile([1, SLAB], F32)
            nc.sync.dma_start(out=valid_sb, in_=slab_valid[:])
            ident_f = cst.tile([128, 128], F32)
            make_identity(nc, ident_f)
            ident = cst.tile([128, 128], F32R)
            nc.vector.tensor_copy(out=ident, in_=ident_f)
            ones_f = cst.tile([128, NJT], F32)
            nc.vector.memset(ones_f, 1.0)

            # halo base registers (Pool engine, persistent)
            hb_sb = cst.tile([1, 4], I32)
            nc.sync.dma_start(out=hb_sb, in_=halo_bases[:])
            halo_vals = []
            for i in range(4):
                reg = nc.alloc_registers(f"halo_reg{i}",
                                         engines=[mybir.EngineType.Pool])
                nc.reg_load(list(reg), hb_sb[0:1, i:i + 1])
                halo_vals.append(nc.snap(reg, donate=False))

            # per-round DRAM buffers
            def dram_tiles():
                out = []
                for rnd in range(ROUNDS):
                    t = {}
                    t["aga_in"] = dr.tile([512, 120], BF16, tag="aga_in", bufs=2,
                                          name=f"aga_in_{rnd}")
                    t["aga_out"] = dr.tile([512 * NCORE, 120], BF16,
                                           addr_space="Shared", tag="aga_out",
                                           bufs=2, name=f"aga_out_{rnd}")
                    t["agb_in"] = dr.tile([1024, 120], BF16, tag="agb_in", bufs=2,
                                          name=f"agb_in_{rnd}")
                    t["agb_out"] = dr.tile([1024 * NCORE, 120], BF16,
                                           addr_space="Shared", tag="agb_out",
                                           bufs=2, name=f"agb_out_{rnd}")
                    t["h_local"] = dr.tile([3, 2, 128, SLAB], F32, tag="h_local",
                                           bufs=2, name=f"h_local_{rnd}")
                    if rnd < ROUNDS - 1:
                        for f in range(3):
                            t[f"agh_in{f}"] = dr.tile(
                                [256, SLAB], F32, tag=f"agh_in{f}", bufs=2,
                                name=f"agh_in{f}_{rnd}")
                            t[f"agh_out{f}"] = dr.tile(
                                [256 * NCORE, SLAB], F32, addr_space="Shared",
                                tag=f"agh_out{f}", bufs=2,
                                name=f"agh_out{f}_{rnd}")
                    out.append(t)
                return out

            DT = dram_tiles()

            for rnd in range(ROUNDS):
                att_bf = {}   # att idx -> bf16 [128, 2, SLAB] tile

                for (qf, att_ids) in PAIRS:
                    # ---------- pre-phase: load Q, build QT + g ----------
                    qfull = qfp.tile([128, 2, D], F32R, tag="qfull",
                                     name=f"qfull_{rnd}_{qf}")
                    if rnd == 0:
                        for et in range(2):
                            nc.sync.dma_start(out=qfull[:, et, :],
                                              in_=featQ[qf, et, :, :].bitcast(F32R))
                    else:
                        src = DT[rnd - 1][f"agh_out{qf}"]
                        for b in range(NCORE):
                            lo = b * SLAB
                            hi = min(lo + SLAB, D)
                            if hi <= lo:
                                continue
                            for et in range(2):
                                nc.sync.dma_start(
                                    out=qfull[:, et, lo:hi],
                                    in_=src[b * 256 + et * 128:
                                            b * 256 + et * 128 + 128,
                                            0:hi - lo].bitcast(F32R))

                    qt = qtp.tile([128, NJT, 256], F32R, tag="qt",
                                  name=f"qt_{rnd}_{qf}")
                    sg = sgp.tile([128, NJT, 2], F32R, tag="sg",
                                  name=f"sg_{rnd}_{qf}")
                    nc.vector.tensor_copy(out=sg[:, :, 0], in_=ones_f)
                    for jt, (js, je) in enumerate(JT):
                        jsz = je - js
                        for et in range(2):
                            tp = ps.tile([128, 128], F32R, tag="big",
                                         bufs=3, name=f"tp_{rnd}_{qf}_{jt}_{et}")
                            nc.tensor.matmul(tp[:jsz, :],
                                             qfull[:, et, js:je],
                                             ident[:], is_transpose=True,
                                             start=True, stop=True)
                            nc.any.tensor_copy(
                                out=qt[:jsz, jt, et * 128:(et + 1) * 128],
                                in_=tp[:jsz, :])
                        gp = ps.tile([128, 4], F32, tag="big", bufs=3,
                                     name=f"gp_{rnd}_{qf}_{jt}")
                        for kt in range(2):
                            nc.tensor.matmul(gp[:jsz, :],
                                             qfull[:, kt, js:je],
                                             wgate_sb[:, kt, :],
                                             start=(kt == 0), stop=(kt == 1))
                        nc.any.tensor_copy(out=sg[:jsz, jt, 1:2], in_=gp[:jsz, 0:1])

                    # ---------- corr_T for both atts ----------
                    corrs = []
                    for ai in att_ids:
                        e = ATTS[ai][0]
                        esl = eslp.tile([128, 2, SLAB], F32R, tag="esl",
                                        name=f"esl_{rnd}_{ai}")
                        for et in range(2):
                            if rnd == 0:
                                nc.sync.dma_start(out=esl[:, et, :],
                                                  in_=feat_slab[e, et, :, :].bitcast(F32R))
                            else:
                                nc.sync.dma_start(
                                    out=esl[:, et, :],
                                    in_=DT[rnd - 1]["h_local"][e, et, :, :].bitcast(F32R))
                        csb = crp.tile([128, 2, SLAB], F32R, tag="corrT",
                                       name=f"csb_{rnd}_{ai}")
                        for eo in range(2):
                            pc = ps.tile([128, SLAB], F32, tag="big", bufs=3,
                                         name=f"pc_{rnd}_{ai}_{eo}")
                            for kt in range(2):
                                nc.tensor.matmul(
                                    pc, wlin_sb[:, kt, eo * 128:(eo + 1) * 128],
                                    esl[:, kt, :],
                                    start=(kt == 0), stop=(kt == 1))
                            nc.any.tensor_copy(out=csb[:, eo, :], in_=pc)
                        corrs.append(csb)

                    # ---------- j-loop ----------
                    att_ps = []
                    sums_acc = []
                    for k, ai in enumerate(att_ids):
                        for ctt in range(2):
                            att_ps.append(ps.tile(
                                [128, SLAB], F32, tag="acc", bufs=4,
                                name=f"attps_{rnd}_{ai}_{ctt}"))
                        sums_acc.append(vecp.tile(
                            [2, SLAB], F32, tag="vec", name=f"sums_{rnd}_{ai}"))
                    for jt, (js, je) in enumerate(JT):
                        jsz = je - js
                        for k, ai in enumerate(att_ids):
                            ap = ps.tile([128, SLAB], F32, tag="big", bufs=3,
                                         name=f"ap_{rnd}_{ai}_{jt}")
                            for kt in range(2):
                                nc.tensor.matmul(ap[:jsz, :],
                                                 qfull[:, kt, js:je],
                                                 corrs[k][:, kt, :],
                                                 start=(kt == 0), stop=(kt == 1))
                            eb = epp.tile([128, SLAB], F32R, tag="ep",
                                          name=f"eb_{rnd}_{ai}_{jt}")
                            nc.scalar.activation(eb[:jsz, :], ap[:jsz, :], AF.Exp)
                            sp = ps.tile([2, SLAB], F32, tag="big", bufs=3,
                                         name=f"sp_{rnd}_{ai}_{jt}")
                            nc.tensor.matmul(sp, sg[:jsz, jt, :],
                                             eb[:jsz, :],
                                             start=True, stop=True)
                            if jt == 0:
                                nc.vector.tensor_copy(out=sums_acc[k], in_=sp)
                            else:
                                nc.vector.tensor_add(out=sums_acc[k],
                                                     in0=sums_acc[k], in1=sp)
                            for ctt in range(2):
                                nc.tensor.matmul(
                                    att_ps[k * 2 + ctt],
                                    qt[:jsz, jt, ctt * 128:(ctt + 1) * 128],
                                    eb[:jsz, :],
                                    start=(jt == 0), stop=(jt == NJT - 1))

                    # ---------- epilogue per att ----------
                    for k, ai in enumerate(att_ids):
                        recip = vecp.tile([2, SLAB], F32, tag="vec",
                                          name=f"recip_{rnd}_{ai}")
                        nc.vector.reciprocal(recip[0:1, :], sums_acc[k][0:1, :])
                        gr0 = vecp.tile([2, SLAB], F32, tag="vec",
                                        name=f"gr0_{rnd}_{ai}")
                        nc.sync.dma_start(out=gr0[0:1, :],
                                          in_=sums_acc[k][1:2, :])
                        scv = vecp.tile([2, SLAB], F32, tag="vec",
                                        name=f"scv_{rnd}_{ai}")
                        nc.vector.tensor_mul(out=scv[0:1, :], in0=gr0[0:1, :],
                                             in1=recip[0:1, :])
                        nc.scalar.activation(scv[0:1, :], scv[0:1, :], AF.Sigmoid)
                        nc.vector.tensor_mul(out=scv[0:1, :], in0=scv[0:1, :],
                                             in1=recip[0:1, :])
                        nc.vector.tensor_mul(out=scv[0:1, :], in0=scv[0:1, :],
                                             in1=valid_sb[0:1, :])
                        scd = dr.tile([1, SLAB], F32, tag="scvd", bufs=2,
                                      name=f"scd_{rnd}_{ai}")
                        nc.sync.dma_start(out=scd, in_=scv[0:1, :])
                        scb = scbp.tile([128, SLAB], F32, tag="scb",
                                        name=f"scb_{rnd}_{ai}")
                        nc.sync.dma_start(out=scb,
                                          in_=scd[0:1, :].partition_broadcast(128))
                        abf = attp.tile([128, 2, SLAB], BF16, tag="attbf",
                                        name=f"abf_{rnd}_{ai}")
                        for ctt in range(2):
                            nc.vector.tensor_tensor(out=abf[:, ctt, :],
                                                    in0=att_ps[k * 2 + ctt],
                                                    in1=scb, op=MUL)
                        att_bf[ai] = abf
                        # edge writes into the AG bounce this att belongs to
                        if ai in AG_A_ATTS:
                            bounce, loc = DT[rnd]["aga_in"], AG_A_ATTS.index(ai)
                        else:
                            bounce, loc = DT[rnd]["agb_in"], AG_B_ATTS.index(ai)
                        for et in range(2):
                            row = loc * 256 + et * 128
                            nc.sync.dma_start(out=bounce[row:row + 128, 0:60],
                                              in_=abf[:, et, 0:60])
                            nc.sync.dma_start(out=bounce[row:row + 128, 60:120],
                                              in_=abf[:, et, SLAB - 60:SLAB])

                    # fire edge collectives at pair boundaries
                    if qf == 2:  # after pair2 (atts 0..3 done; AG-a atts ready)
                        nc.gpsimd.collective_compute(
                            "AllGather", mybir.AluOpType.bypass,
                            replica_groups=[list(range(NCORE))],
                            ins=[DT[rnd]["aga_in"][:].opt()],
                            outs=[DT[rnd]["aga_out"][:].opt()])
                    if qf == 0:  # after pair3
                        nc.gpsimd.collective_compute(
                            "AllGather", mybir.AluOpType.bypass,
                            replica_groups=[list(range(NCORE))],
                            ins=[DT[rnd]["agb_in"][:].opt()],
                            outs=[DT[rnd]["agb_out"][:].opt()])

                # ---------- convs + GRUs ----------
                for d in range(3):
                    pa, pb = CONV_PARTS[d]
                    inp = padp.tile([128, 4, 622], BF16, tag="inpad",
                                    name=f"inp_{rnd}_{d}")
                    nc.vector.memset(inp, 0.0)
                    for part, ai in enumerate((pa, pb)):
                        for et in range(2):
                            kt = part * 2 + et
                            # own tokens at cols 64 + 62*row
                            dst = inp[:, kt, 64:64 + 8 * PW].rearrange(
                                "p (r w) -> p r w", w=PW)[:, :, 0:HW]
                            src = att_bf[ai][:, et, :].rearrange(
                                "p (r w) -> p r w", w=HW)
                            nc.sync.dma_start(out=dst, in_=src)
                            # halos
                            if ai in AG_A_ATTS:
                                agout = DT[rnd]["aga_out"]
                                loc = AG_A_ATTS.index(ai)
                                lval, rval = halo_vals[0], halo_vals[1]
                            else:
                                agout = DT[rnd]["agb_out"]
                                loc = AG_B_ATTS.index(ai)
                                lval, rval = halo_vals[2], halo_vals[3]
                            row = loc * 256 + et * 128
                            nc.gpsimd.dma_start(
                                out=inp[:, kt, 2:62],
                                in_=agout[row:][bass.ds(lval, 128), 60:120])
                            nc.vector.tensor_scalar_mul(
                                out=inp[:, kt, 2:62], in0=inp[:, kt, 2:62],
                                scalar1=hmask_sb[:, 0:1])
                            nc.gpsimd.dma_start(
                                out=inp[:, kt, 560:620],
                                in_=agout[row:][bass.ds(rval, 128), 0:60])
                            nc.vector.tensor_scalar_mul(
                                out=inp[:, kt, 560:620], in0=inp[:, kt, 560:620],
                                scalar1=hmask_sb[:, 1:2])

                    a_sb = asbp.tile([128, 2, SLAB], F32R, tag="asb",
                                     name=f"asb_{rnd}_{d}")
                    for ctt in range(2):
                        cp = ps.tile([128, 497], F32, tag="conv", bufs=1,
                                     name=f"cp_{rnd}_{d}_{ctt}")
                        first = True
                        for kt in range(4):
                            for ky in range(3):
                                for kx in range(3):
                                    dpp = (ky - 1) * PW + (kx - 1)
                                    nc.tensor.matmul(
                                        cp[:, 0:496],
                                        wcf_sb[:, ky * 3 + kx, kt,
                                               ctt * 128:(ctt + 1) * 128],
                                        inp[:, kt, 63 + dpp:63 + dpp + 496],
                                        start=first,
                                        stop=(kt == 3 and ky == 2 and kx == 2))
                                    first = False
                        cpx = cp[:, 1:1 + 8 * PW].rearrange(
                            "p (r w) -> p r w", w=PW)[:, :, 0:HW]
                        nc.vector.tensor_scalar_add(
                            out=a_sb[:, ctt, :].rearrange("p (r w) -> p r w", w=HW),
                            in0=cpx, scalar1=bcf_sb[:, ctt:ctt + 1])

                    # ---- GRU d ----
                    prev = prvp.tile([128, 2, SLAB], F32R, tag="prev",
                                     name=f"prev_{rnd}_{d}")
                    for et in range(2):
                        if rnd == 0:
                            nc.sync.dma_start(out=prev[:, et, :],
                                              in_=feat_slab[d, et, :, :].bitcast(F32R))
                        else:
                            nc.sync.dma_start(
                                out=prev[:, et, :],
                                in_=DT[rnd - 1]["h_local"][d, et, :, :].bitcast(F32R))

                    def gate1x1(gate_i, rhs_pairs, func, outname):
                        gt = grup.tile([128, 2, SLAB], F32, tag="grutmp",
                                       name=outname)
                        for ctt in range(2):
                            gps = ps.tile([128, SLAB], F32, tag="conv", bufs=1,
                                          name=f"{outname}_ps{ctt}")
                            for kt in range(4):
                                nc.tensor.matmul(
                                    gps,
                                    gruw_sb[:, gate_i, kt,
                                                ctt * 128:(ctt + 1) * 128],
                                    rhs_pairs[kt],
                                    start=(kt == 0), stop=(kt == 3))
                            nc.scalar.activation(
                                gt[:, ctt, :], gps, func,
                                bias=grub_sb[:, gate_i, ctt:ctt + 1])
                        return gt

                    st = [a_sb[:, 0, :], a_sb[:, 1, :], prev[:, 0, :],
                          prev[:, 1, :]]
                    # gru_W order: 0=reset, 1=update, 2=out
                    u = gate1x1(1, st, AF.Sigmoid, f"u_{rnd}_{d}")
                    rg = gate1x1(0, st, AF.Sigmoid, f"r_{rnd}_{d}")
                    pr = grup.tile([128, 2, SLAB], F32R, tag="grutmp",
                                   name=f"pr_{rnd}_{d}")
                    for ctt in range(2):
                        nc.vector.tensor_mul(out=pr[:, ctt, :],
                                             in0=prev[:, ctt, :],
                                             in1=rg[:, ctt, :])
                    st2 = [a_sb[:, 0, :], a_sb[:, 1, :], pr[:, 0, :], pr[:, 1, :]]
                    o = gate1x1(2, st2, AF.Tanh, f"o_{rnd}_{d}")
                    h = hp.tile([128, 2, SLAB], F32, tag="h", name=f"h_{rnd}_{d}")
                    for ctt in range(2):
                        # h = prev + u * (o - prev)
                        nc.vector.tensor_sub(out=o[:, ctt, :], in0=o[:, ctt, :],
                                             in1=prev[:, ctt, :])
                        nc.vector.tensor_mul(out=o[:, ctt, :], in0=o[:, ctt, :],
                                             in1=u[:, ctt, :])
                        nc.vector.tensor_add(out=h[:, ctt, :],
                                             in0=prev[:, ctt, :],
                                             in1=o[:, ctt, :])
                    for et in range(2):
                        nc.sync.dma_start(out=DT[rnd]["h_local"][d, et, :, :],
                                          in_=h[:, et, :])
                        if rnd == ROUNDS - 1:
                            nc.sync.dma_start(out=out_slab[d, et, :, :],
                                              in_=h[:, et, :])
                        else:
                            nc.sync.dma_start(
                                out=DT[rnd][f"agh_in{d}"][et * 128:et * 128 + 128, :],
                                in_=h[:, et, :])
                    if rnd < ROUNDS - 1:
                        nc.gpsimd.collective_compute(
                            "AllGather", mybir.AluOpType.bypass,
                            replica_groups=[list(range(NCORE))],
                            ins=[DT[rnd][f"agh_in{d}"][:].opt()],
                            outs=[DT[rnd][f"agh_out{d}"][:].opt()])

    nc.compile()
    return nc


_NC_CACHE = None


def _get_nc():
    global _NC_CACHE
    if _NC_CACHE is None:
        _NC_CACHE = _build_nc()
    return _NC_CACHE


def _prep_inputs(inputs):
    f32 = np.float32
    feats = [np.ascontiguousarray(np.asarray(inputs[k], f32).reshape(C, D))
             for k in ("infeature1", "infeature2", "infeature3")]
    featQ = np.stack([f.reshape(2, 128, D) for f in feats])  # [3,2,128,D]

    W_lin = np.asarray(inputs["W_lin"], f32)
    W_linT = np.ascontiguousarray(W_lin.T.reshape(2, 128, 256))
    W_gate = np.zeros((2, 128, 4), f32)
    W_gate[:, :, 0] = np.asarray(inputs["W_gate"], f32).reshape(2, 128)
    W_cf = np.asarray(inputs["W_cf"], f32)
    W_cfT = np.ascontiguousarray(
        W_cf.transpose(2, 3, 1, 0).reshape(9, 512, 256).reshape(9, 4, 128, 256)
    ).astype(ml_dtypes.bfloat16)
    b_cf2 = np.asarray(inputs["b_cf"], f32).reshape(2, 128)
    gru_W = np.stack([
        np.ascontiguousarray(np.asarray(inputs[k], f32).T.reshape(4, 128, 256))
        for k in ("W_reset", "W_update", "W_out")])
    gru_b = np.stack([np.asarray(inputs[k], f32).reshape(2, 128)
                      for k in ("b_reset", "b_update", "b_out")])

    common = dict(featQ=featQ, W_linT=W_linT, W_gate_r=W_gate, W_cfT=W_cfT,
                  b_cf2=b_cf2, gru_W=gru_W, gru_b=gru_b)

    in_maps = []
    for r in range(NCORE):
        t0 = r * SLAB
        fs = np.zeros((3, 2, 128, SLAB), f32)
        n = max(0, min(t0 + SLAB, D) - t0)
        if n > 0:
            fs[:, :, :, :n] = featQ[:, :, :, t0:t0 + n]
        valid = np.zeros((1, SLAB), f32)
        valid[0, :n] = 1.0
        hb = np.array([[((r + 7) % 8) * 512, ((r + 1) % 8) * 512,
                        ((r + 7) % 8) * 1024, ((r + 1) % 8) * 1024]], np.int32)
        hm = np.zeros((128, 2), f32)
        hm[:, 0] = 0.0 if r == 0 else 1.0
        hm[:, 1] = 0.0 if r == NCORE - 1 else 1.0
        m = dict(common)
        m.update(feat_slab=fs, halo_bases=hb, halo_mask=hm, slab_valid=valid)
        in_maps.append(m)
    return in_maps


def kernel(**inputs):
    nc = _get_nc()
    in_maps = _prep_inputs(inputs)
    res = run_bass_kernel_spmd(nc, in_maps, core_ids=list(range(NCORE)))
    outs = []
    for f in range(3):
        full = np.zeros((C, D), np.float32)
        for r in range(NCORE):
            t0 = r * SLAB
            n = max(0, min(t0 + SLAB, D) - t0)
            if n > 0:
                sl = res.results[r]["out_slab"][f].reshape(C, SLAB)
                full[:, t0:t0 + n] = sl[:, :n]
        outs.append(full.reshape(1, C, HW, HW))
    return tuple(outs)


if __name__ == "__main__":
    # build-only check
    nc = _get_nc()
    print("build OK")



# revision 2
# speedup vs baseline: 11.9762x; 11.9762x over previous
"""Trainium2 Bass kernel for nn_CoattentionModel (co-attention + conv-fusion + convGRU).

Sharding: token axis (3600 tokens = 60x60 image) padded to 64 rows (3840 tokens),
split 8 ways -> each core owns 8 image rows (480 tokens). Attention is computed
as A'[j,i] tiles (query-token j on partitions), softmax without max-subtraction
(logits verified <= ~40), attention output accumulated over 29 j-tiles in PSUM.
Softmax sum + gate row come from a 2-row matmul against [ones | g] per j-tile.
Normalize * sigmoid-gate * pad-valid mask fold into one per-column scale vector.
Matmuls run in float32r (full PE rate, ~1e-3 max rel err); the 3x3 conv path
runs in bf16 to fit SBUF. Per round: 2 edge AllGathers provide conv halos
(read back at rank-dynamic register offsets), 3 feature AllGathers rebuild the
full features for the next round's attention.

Host I/O strategy (the axon relay charges ~50-150ms per transfer + ~60MB/s):
ship ONE f32 array per core [1, PACK_N] holding that core's feature slab plus
1/8th of the replicated weights; the kernel AllGathers the weight shards and
the feature slabs on device. Output returns as f16 (halves download bytes).
The jitted shard_map executable is built once and cached — per-call cost is
upload + execute + download only. This is the same _bass_exec_p/PJRT path
run_bass_kernel_spmd takes under axon, minus its per-call retrace.
"""
import sys
for _p in ("/opt/trn_rl_repo", "/root/.axon_site/_ro/trn_rl_repo"):
    if _p not in sys.path:
        sys.path.insert(0, _p)

import numpy as np

import concourse.bass as bass
import concourse.mybir as mybir
import concourse.tile as tile
from concourse import bacc
from concourse.masks import make_identity

F32 = mybir.dt.float32
F32R = mybir.dt.float32r
F16 = mybir.dt.float16
BF16 = mybir.dt.bfloat16
I32 = mybir.dt.int32
AF = mybir.ActivationFunctionType
MUL = mybir.AluOpType.mult

C = 256
HW = 60
D = HW * HW              # 3600
ROWS_PAD = 64
D_PAD = ROWS_PAD * HW    # 3840
NCORE = 8
SLAB = D_PAD // NCORE    # 480
PW = HW + 2              # padded image width
ROUNDS = 5
JT = [(s, min(s + 128, D)) for s in range(0, D, 128)]  # 29 j-tiles over REAL tokens
NJT = len(JT)

# attention list: (E feature, Q feature), grouped in pairs sharing Q
ATTS = [(0, 1), (2, 1), (0, 2), (1, 2), (1, 0), (2, 0)]
PAIRS = [(1, [0, 1]), (2, [2, 3]), (0, [4, 5])]  # (Q feature, att indices)
# conv d consumes (attA, attB) channel-concat; GRU prev = feature d
CONV_PARTS = [(0, 2), (4, 3), (5, 1)]
# edge AllGather membership: AG-a = atts {0, 2} (ready after pair2) -> conv1
#                            AG-b = atts {1, 3, 4, 5} -> conv2, conv3
AG_A_ATTS = [0, 2]
AG_B_ATTS = [1, 3, 4, 5]

# ---------------- packed-input layout (f32 element offsets) ----------------
# Per-core section S:
OF_FEAT = 0                      # [3, 2, 128, SLAB] own feature slab
LEN_FEAT = 3 * 2 * 128 * SLAB    # 368640  (= 720 * 512)
OF_VALID = 368640                # [1, SLAB] valid-token mask
OF_HMASK = 369152                # [128, 2] halo edge mask
OF_HB = 369664                   # [1, 4] halo base rows (int32 bits)
S_LEN = 370176                   # 723 * 512
# Replicated-weight pack W (each core uploads 1/8, AllGather rebuilds):
WOF_LIN = 0                      # [2, 128, 256] W_lin^T
WOF_GATE = 65536                 # [2, 128, 4]   gate weight (col 0)
WOF_BCF = 66560                  # [2, 128]      conv bias
WOF_GRUB = 67072                 # [3, 2, 128]   GRU biases
WOF_CF = 68096                   # [9, 4, 128, 256] conv_fusion taps (f32; cast
                                 #                  to bf16 on device)
WOF_GRUW = 1247744               # [3, 4, 128, 256] GRU weights
W_LEN = 1642496                  # 401 * 4096 (padded to 8*512 multiple)
W_ROWS_PC = 401                  # weight-shard rows ([512-wide]) per core
WPC = W_ROWS_PC * 512            # 205312
PACK_N = S_LEN + WPC             # 575488


def r32(ap):
    return ap.bitcast(F32R)


def _build_nc():
    nc = bacc.Bacc("TRN2", target_bir_lowering=False, debug=False,
                   num_devices=NCORE)

    pack = nc.dram_tensor("pack", [1, PACK_N], F32, kind="ExternalInput")
    out_slab = nc.dram_tensor("out_slab", [3, 2, 128, SLAB], F16,
                              kind="ExternalOutput")

    with tile.TileContext(nc) as tc:
        import contextlib
        ctx = contextlib.ExitStack()
        with ctx:
            cst = ctx.enter_context(tc.tile_pool(name="cst", bufs=1))
            qfp = ctx.enter_context(tc.tile_pool(name="qfp", bufs=1))
            qtp = ctx.enter_context(tc.tile_pool(name="qtp", bufs=1))
            sgp = ctx.enter_context(tc.tile_pool(name="sgp", bufs=1))
            eslp = ctx.enter_context(tc.tile_pool(name="eslp", bufs=2))
            crp = ctx.enter_context(tc.tile_pool(name="crp", bufs=2))
            epp = ctx.enter_context(tc.tile_pool(name="epp", bufs=4))
            attp = ctx.enter_context(tc.tile_pool(name="attp", bufs=8))
            vecp = ctx.enter_context(tc.tile_pool(name="vecp", bufs=6))
            scbp = ctx.enter_context(tc.tile_pool(name="scbp", bufs=2))
            padp = ctx.enter_context(tc.tile_pool(name="padp", bufs=1))
            asbp = ctx.enter_context(tc.tile_pool(name="asbp", bufs=2))
            prvp = ctx.enter_context(tc.tile_pool(name="prvp", bufs=2))
            grup = ctx.enter_context(tc.tile_pool(name="grup", bufs=3))
            hp = ctx.enter_context(tc.tile_pool(name="hp", bufs=2))
            ps = ctx.enter_context(tc.tile_pool(name="ps", bufs=1, space="PSUM"))
            dr = ctx.enter_context(tc.tile_pool(name="dr", bufs=1, space="DRAM"))

            # ---- rebuild replicated weights + full features on device ----
            wag_in = dr.tile([W_ROWS_PC, 512], F32, name="wag_in")
            nc.sync.dma_start(
                out=wag_in,
                in_=pack[0, S_LEN:PACK_N].rearrange("(r c) -> r c", c=512))
            wag_out = dr.tile([W_ROWS_PC * NCORE, 512], F32,
                              addr_space="Shared", name="wag_out")
            nc.gpsimd.collective_compute(
                "AllGather", mybir.AluOpType.bypass,
                replica_groups=[list(range(NCORE))],
                ins=[wag_in[:].opt()], outs=[wag_out[:].opt()])

            fag_in = dr.tile([768, SLAB], F32, name="fag_in")
            nc.sync.dma_start(
                out=fag_in,
                in_=pack[0, OF_FEAT:OF_FEAT + LEN_FEAT].rearrange(
                    "(r s) -> r s", s=SLAB))
            fag_out = dr.tile([768 * NCORE, SLAB], F32, addr_space="Shared",
                              name="fag_out")
            nc.gpsimd.collective_compute(
                "AllGather", mybir.AluOpType.bypass,
                replica_groups=[list(range(NCORE))],
                ins=[fag_in[:].opt()], outs=[fag_out[:].opt()])

            wflat = wag_out[:].rearrange("r c -> (r c)")

            def wsl(ofs, n):
                return wflat[ofs:ofs + n]

            # ------------- constants -------------
            wlin_sb = cst.tile([128, 2, 256], F32R)
            nc.sync.dma_start(
                out=wlin_sb,
                in_=wsl(WOF_LIN, 65536).rearrange(
                    "(k p e) -> p k e", k=2, p=128).bitcast(F32R))
            wgate_sb = cst.tile([128, 2, 4], F32R)
            nc.sync.dma_start(
                out=wgate_sb,
                in_=wsl(WOF_GATE, 1024).rearrange(
                    "(k p n) -> p k n", k=2, p=128).bitcast(F32R))
            bcf_sb = cst.tile([128, 2], F32)
            nc.sync.dma_start(
                out=bcf_sb,
                in_=wsl(WOF_BCF, 256).rearrange("(c p) -> p c", c=2))
            grub_sb = cst.tile([128, 3, 2], F32)
            nc.sync.dma_start(
                out=grub_sb,
                in_=wsl(WOF_GRUB, 768).rearrange("(g c p) -> p g c", g=3, c=2))
            gruw_sb = cst.tile([128, 3, 4, 256], F32R)
            nc.sync.dma_start(
                out=gruw_sb,
                in_=wsl(WOF_GRUW, 393216).rearrange(
                    "(g k p o) -> p g k o", g=3, k=4, p=128).bitcast(F32R))
            # conv taps arrive f32; stage per-tap and cast to bf16 (SBUF-cheap)
            wcf_sb = cst.tile([128, 9, 4, 256], BF16)
            wcfstg = ctx.enter_context(tc.tile_pool(name="wcfstg", bufs=2))
            for t in range(9):
                stg = wcfstg.tile([128, 4, 256], F32, tag="stg",
                                  name=f"wcfstg_{t}")
                nc.sync.dma_start(
                    out=stg,
                    in_=wsl(WOF_CF + t * 131072, 131072).rearrange(
                        "(k p o) -> p k o", k=4, p=128))
                nc.vector.tensor_copy(out=wcf_sb[:, t, :, :], in_=stg)
            hmask_sb = cst.tile([128, 2], F32)
            nc.sync.dma_start(
                out=hmask_sb,
                in_=pack[0, OF_HMASK:OF_HMASK + 256].rearrange(
                    "(p c) -> p c", c=2))
            valid_sb = cst.tile([1, SLAB], F32)
            nc.sync.dma_start(
                out=valid_sb,
                in_=pack[0, OF_VALID:OF_VALID + SLAB].rearrange(
                    "(a s) -> a s", a=1))
            ident_f = cst.tile([128, 128], F32)
            make_identity(nc, ident_f)
            ident = cst.tile([128, 128], F32R)
            nc.vector.tensor_copy(out=ident, in_=ident_f)
            ones_f = cst.tile([128, NJT], F32)
            nc.vector.memset(ones_f, 1.0)

            # halo base registers (Pool engine, persistent)
            hb_sb = cst.tile([1, 4], I32)
            nc.sync.dma_start(
                out=hb_sb,
                in_=pack[0, OF_HB:OF_HB + 4].rearrange(
                    "(a b) -> a b", a=1).bitcast(I32))
            halo_vals = []
            for i in range(4):
                reg = nc.alloc_registers(f"halo_reg{i}",
                                         engines=[mybir.EngineType.Pool])
                nc.reg_load(list(reg), hb_sb[0:1, i:i + 1])
                halo_vals.append(nc.snap(reg, donate=False))

            def own_feat(f, et):
                ofs = OF_FEAT + (f * 2 + et) * 128 * SLAB
                return pack[0, ofs:ofs + 128 * SLAB].rearrange(
                    "(p s) -> p s", s=SLAB)

            # per-round DRAM buffers
            def dram_tiles():
                out = []
                for rnd in range(ROUNDS):
                    t = {}
                    t["aga_in"] = dr.tile([512, 120], BF16, tag="aga_in", bufs=2,
                                          name=f"aga_in_{rnd}")
                    t["aga_out"] = dr.tile([512 * NCORE, 120], BF16,
                                           addr_space="Shared", tag="aga_out",
                                           bufs=2, name=f"aga_out_{rnd}")
                    t["agb_in"] = dr.tile([1024, 120], BF16, tag="agb_in", bufs=2,
                                          name=f"agb_in_{rnd}")
                    t["agb_out"] = dr.tile([1024 * NCORE, 120], BF16,
                                           addr_space="Shared", tag="agb_out",
                                           bufs=2, name=f"agb_out_{rnd}")
                    t["h_local"] = dr.tile([3, 2, 128, SLAB], F32, tag="h_local",
                                           bufs=2, name=f"h_local_{rnd}")
                    if rnd < ROUNDS - 1:
                        for f in range(3):
                            t[f"agh_in{f}"] = dr.tile(
                                [256, SLAB], F32, tag=f"agh_in{f}", bufs=2,
                                name=f"agh_in{f}_{rnd}")
                            t[f"agh_out{f}"] = dr.tile(
                                [256 * NCORE, SLAB], F32, addr_space="Shared",
                                tag=f"agh_out{f}", bufs=2,
                                name=f"agh_out{f}_{rnd}")
                    out.append(t)
                return out

            DT = dram_tiles()

            for rnd in range(ROUNDS):
                att_bf = {}   # att idx -> bf16 [128, 2, SLAB] tile

                for (qf, att_ids) in PAIRS:
                    # ---------- pre-phase: load Q, build QT + g ----------
                    qfull = qfp.tile([128, 2, D], F32R, tag="qfull",
                                     name=f"qfull_{rnd}_{qf}")
                    if rnd == 0:
                        src, rstride, rbase = fag_out, 768, qf * 256
                    else:
                        src, rstride, rbase = DT[rnd - 1][f"agh_out{qf}"], 256, 0
                    for b in range(NCORE):
                        lo = b * SLAB
                        hi = min(lo + SLAB, D)
                        if hi <= lo:
                            continue
                        for et in range(2):
                            row = b * rstride + rbase + et * 128
                            nc.sync.dma_start(
                                out=qfull[:, et, lo:hi],
                                in_=src[row:row + 128, 0:hi - lo].bitcast(F32R))

                    qt = qtp.tile([128, NJT, 256], F32R, tag="qt",
                                  name=f"qt_{rnd}_{qf}")
                    sg = sgp.tile([128, NJT, 2], F32R, tag="sg",
                                  name=f"sg_{rnd}_{qf}")
                    nc.vector.tensor_copy(out=sg[:, :, 0], in_=ones_f)
                    for jt, (js, je) in enumerate(JT):
                        jsz = je - js
                        for et in range(2):
                            tp = ps.tile([128, 128], F32R, tag="big",
                                         bufs=3, name=f"tp_{rnd}_{qf}_{jt}_{et}")
                            nc.tensor.matmul(tp[:jsz, :],
                                             qfull[:, et, js:je],
                                             ident[:], is_transpose=True,
                                             start=True, stop=True)
                            nc.any.tensor_copy(
                                out=qt[:jsz, jt, et * 128:(et + 1) * 128],
                                in_=tp[:jsz, :])
                        gp = ps.tile([128, 4], F32, tag="big", bufs=3,
                                     name=f"gp_{rnd}_{qf}_{jt}")
                        for kt in range(2):
                            nc.tensor.matmul(gp[:jsz, :],
                                             qfull[:, kt, js:je],
                                             wgate_sb[:, kt, :],
                                             start=(kt == 0), stop=(kt == 1))
                        nc.any.tensor_copy(out=sg[:jsz, jt, 1:2], in_=gp[:jsz, 0:1])

                    # ---------- corr_T for both atts ----------
                    corrs = []
                    for ai in att_ids:
                        e = ATTS[ai][0]
                        esl = eslp.tile([128, 2, SLAB], F32R, tag="esl",
                                        name=f"esl_{rnd}_{ai}")
                        for et in range(2):
                            if rnd == 0:
                                nc.sync.dma_start(out=esl[:, et, :],
                                                  in_=own_feat(e, et).bitcast(F32R))
                            else:
                                nc.sync.dma_start(
                                    out=esl[:, et, :],
                                    in_=DT[rnd - 1]["h_local"][e, et, :, :].bitcast(F32R))
                        csb = crp.tile([128, 2, SLAB], F32R, tag="corrT",
                                       name=f"csb_{rnd}_{ai}")
                        for eo in range(2):
                            pc = ps.tile([128, SLAB], F32, tag="big", bufs=3,
                                         name=f"pc_{rnd}_{ai}_{eo}")
                            for kt in range(2):
                                nc.tensor.matmul(
                                    pc, wlin_sb[:, kt, eo * 128:(eo + 1) * 128],
                                    esl[:, kt, :],
                                    start=(kt == 0), stop=(kt == 1))
                            nc.any.tensor_copy(out=csb[:, eo, :], in_=pc)
                        corrs.append(csb)

                    # ---------- j-loop ----------
                    att_ps = []
                    sums_acc = []
                    for k, ai in enumerate(att_ids):
                        for ctt in range(2):
                            att_ps.append(ps.tile(
                                [128, SLAB], F32, tag="acc", bufs=4,
                                name=f"attps_{rnd}_{ai}_{ctt}"))
                        sums_acc.append(vecp.tile(
                            [2, SLAB], F32, tag="vec", name=f"sums_{rnd}_{ai}"))
                    for jt, (js, je) in enumerate(JT):
                        jsz = je - js
                        for k, ai in enumerate(att_ids):
                            ap = ps.tile([128, SLAB], F32, tag="big", bufs=3,
                                         name=f"ap_{rnd}_{ai}_{jt}")
                            for kt in range(2):
                                nc.tensor.matmul(ap[:jsz, :],
                                                 qfull[:, kt, js:je],
                                                 corrs[k][:, kt, :],
                                                 start=(kt == 0), stop=(kt == 1))
                            eb = epp.tile([128, SLAB], F32R, tag="ep",
                                          name=f"eb_{rnd}_{ai}_{jt}")
                            nc.scalar.activation(eb[:jsz, :], ap[:jsz, :], AF.Exp)
                            sp = ps.tile([2, SLAB], F32, tag="big", bufs=3,
                                         name=f"sp_{rnd}_{ai}_{jt}")
                            nc.tensor.matmul(sp, sg[:jsz, jt, :],
                                             eb[:jsz, :],
                                             start=True, stop=True)
                            if jt == 0:
                                nc.vector.tensor_copy(out=sums_acc[k], in_=sp)
                            else:
                                nc.vector.tensor_add(out=sums_acc[k],
                                                     in0=sums_acc[k], in1=sp)
                            for ctt in range(2):
                                nc.tensor.matmul(
                                    att_ps[k * 2 + ctt],
                                    qt[:jsz, jt, ctt * 128:(ctt + 1) * 128],
                                    eb[:jsz, :],
                                    start=(jt == 0), stop=(jt == NJT - 1))

                    # ---------- epilogue per att ----------
                    for k, ai in enumerate(att_ids):
                        recip = vecp.tile([2, SLAB], F32, tag="vec",
                                          name=f"recip_{rnd}_{ai}")
                        nc.vector.reciprocal(recip[0:1, :], sums_acc[k][0:1, :])
                        gr0 = vecp.tile([2, SLAB], F32, tag="vec",
                                        name=f"gr0_{rnd}_{ai}")
                        nc.sync.dma_start(out=gr0[0:1, :],
                                          in_=sums_acc[k][1:2, :])
                        scv = vecp.tile([2, SLAB], F32, tag="vec",
                                        name=f"scv_{rnd}_{ai}")
                        nc.vector.tensor_mul(out=scv[0:1, :], in0=gr0[0:1, :],
                                             in1=recip[0:1, :])
                        nc.scalar.activation(scv[0:1, :], scv[0:1, :], AF.Sigmoid)
                        nc.vector.tensor_mul(out=scv[0:1, :], in0=scv[0:1, :],
                                             in1=recip[0:1, :])
                        nc.vector.tensor_mul(out=scv[0:1, :], in0=scv[0:1, :],
                                             in1=valid_sb[0:1, :])
                        scd = dr.tile([1, SLAB], F32, tag="scvd", bufs=2,
                                      name=f"scd_{rnd}_{ai}")
                        nc.sync.dma_start(out=scd, in_=scv[0:1, :])
                        scb = scbp.tile([128, SLAB], F32, tag="scb",
                                        name=f"scb_{rnd}_{ai}")
                        nc.sync.dma_start(out=scb,
                                          in_=scd[0:1, :].partition_broadcast(128))
                        abf = attp.tile([128, 2, SLAB], BF16, tag="attbf",
                                        name=f"abf_{rnd}_{ai}")
                        for ctt in range(2):
                            nc.vector.tensor_tensor(out=abf[:, ctt, :],
                                                    in0=att_ps[k * 2 + ctt],
                                                    in1=scb, op=MUL)
                        att_bf[ai] = abf
                        # edge writes into the AG bounce this att belongs to
                        if ai in AG_A_ATTS:
                            bounce, loc = DT[rnd]["aga_in"], AG_A_ATTS.index(ai)
                        else:
                            bounce, loc = DT[rnd]["agb_in"], AG_B_ATTS.index(ai)
                        for et in range(2):
                            row = loc * 256 + et * 128
                            nc.sync.dma_start(out=bounce[row:row + 128, 0:60],
                                              in_=abf[:, et, 0:60])
                            nc.sync.dma_start(out=bounce[row:row + 128, 60:120],
                                              in_=abf[:, et, SLAB - 60:SLAB])

                    # fire edge collectives at pair boundaries
                    if qf == 2:  # after pair2 (atts 0..3 done; AG-a atts ready)
                        nc.gpsimd.collective_compute(
                            "AllGather", mybir.AluOpType.bypass,
                            replica_groups=[list(range(NCORE))],
                            ins=[DT[rnd]["aga_in"][:].opt()],
                            outs=[DT[rnd]["aga_out"][:].opt()])
                    if qf == 0:  # after pair3
                        nc.gpsimd.collective_compute(
                            "AllGather", mybir.AluOpType.bypass,
                            replica_groups=[list(range(NCORE))],
                            ins=[DT[rnd]["agb_in"][:].opt()],
                            outs=[DT[rnd]["agb_out"][:].opt()])

                # ---------- convs + GRUs ----------
                for d in range(3):
                    pa, pb = CONV_PARTS[d]
                    inp = padp.tile([128, 4, 622], BF16, tag="inpad",
                                    name=f"inp_{rnd}_{d}")
                    nc.vector.memset(inp, 0.0)
                    for part, ai in enumerate((pa, pb)):
                        for et in range(2):
                            kt = part * 2 + et
                            # own tokens at cols 64 + 62*row
                            dst = inp[:, kt, 64:64 + 8 * PW].rearrange(
                                "p (r w) -> p r w", w=PW)[:, :, 0:HW]
                            src = att_bf[ai][:, et, :].rearrange(
                                "p (r w) -> p r w", w=HW)
                            nc.sync.dma_start(out=dst, in_=src)
                            # halos
                            if ai in AG_A_ATTS:
                                agout = DT[rnd]["aga_out"]
                                loc = AG_A_ATTS.index(ai)
                                lval, rval = halo_vals[0], halo_vals[1]
                            else:
                                agout = DT[rnd]["agb_out"]
                                loc = AG_B_ATTS.index(ai)
                                lval, rval = halo_vals[2], halo_vals[3]
                            row = loc * 256 + et * 128
                            nc.gpsimd.dma_start(
                                out=inp[:, kt, 2:62],
                                in_=agout[row:][bass.ds(lval, 128), 60:120])
                            nc.vector.tensor_scalar_mul(
                                out=inp[:, kt, 2:62], in0=inp[:, kt, 2:62],
                                scalar1=hmask_sb[:, 0:1])
                            nc.gpsimd.dma_start(
                                out=inp[:, kt, 560:620],
                                in_=agout[row:][bass.ds(rval, 128), 0:60])
                            nc.vector.tensor_scalar_mul(
                                out=inp[:, kt, 560:620], in0=inp[:, kt, 560:620],
                                scalar1=hmask_sb[:, 1:2])

                    a_sb = asbp.tile([128, 2, SLAB], F32R, tag="asb",
                                     name=f"asb_{rnd}_{d}")
                    for ctt in range(2):
                        cp = ps.tile([128, 497], F32, tag="conv", bufs=1,
                                     name=f"cp_{rnd}_{d}_{ctt}")
                        first = True
                        for kt in range(4):
                            for ky in range(3):
                                for kx in range(3):
                                    dpp = (ky - 1) * PW + (kx - 1)
                                    nc.tensor.matmul(
                                        cp[:, 0:496],
                                        wcf_sb[:, ky * 3 + kx, kt,
                                               ctt * 128:(ctt + 1) * 128],
                                        inp[:, kt, 63 + dpp:63 + dpp + 496],
                                        start=first,
                                        stop=(kt == 3 and ky == 2 and kx == 2))
                                    first = False
                        cpx = cp[:, 1:1 + 8 * PW].rearrange(
                            "p (r w) -> p r w", w=PW)[:, :, 0:HW]
                        nc.vector.tensor_scalar_add(
                            out=a_sb[:, ctt, :].rearrange("p (r w) -> p r w", w=HW),
                            in0=cpx, scalar1=bcf_sb[:, ctt:ctt + 1])

                    # ---- GRU d ----
                    prev = prvp.tile([128, 2, SLAB], F32R, tag="prev",
                                     name=f"prev_{rnd}_{d}")
                    for et in range(2):
                        if rnd == 0:
                            nc.sync.dma_start(out=prev[:, et, :],
                                              in_=own_feat(d, et).bitcast(F32R))
                        else:
                            nc.sync.dma_start(
                                out=prev[:, et, :],
                                in_=DT[rnd - 1]["h_local"][d, et, :, :].bitcast(F32R))

                    def gate1x1(gate_i, rhs_pairs, func, outname):
                        gt = grup.tile([128, 2, SLAB], F32, tag="grutmp",
                                       name=outname)
                        for ctt in range(2):
                            gps = ps.tile([128, SLAB], F32, tag="conv", bufs=1,
                                          name=f"{outname}_ps{ctt}")
                            for kt in range(4):
                                nc.tensor.matmul(
                                    gps,
                                    gruw_sb[:, gate_i, kt,
                                                ctt * 128:(ctt + 1) * 128],
                                    rhs_pairs[kt],
                                    start=(kt == 0), stop=(kt == 3))
                            nc.scalar.activation(
                                gt[:, ctt, :], gps, func,
                                bias=grub_sb[:, gate_i, ctt:ctt + 1])
                        return gt

                    st = [a_sb[:, 0, :], a_sb[:, 1, :], prev[:, 0, :],
                          prev[:, 1, :]]
                    # gru_W order: 0=reset, 1=update, 2=out
                    u = gate1x1(1, st, AF.Sigmoid, f"u_{rnd}_{d}")
                    rg = gate1x1(0, st, AF.Sigmoid, f"r_{rnd}_{d}")
                    pr = grup.tile([128, 2, SLAB], F32R, tag="grutmp",
                                   name=f"pr_{rnd}_{d}")
                    for ctt in range(2):
                        nc.vector.tensor_mul(out=pr[:, ctt, :],
                                             in0=prev[:, ctt, :],
                                             in1=rg[:, ctt, :])
                    st2 = [a_sb[:, 0, :], a_sb[:, 1, :], pr[:, 0, :], pr[:, 1, :]]
                    o = gate1x1(2, st2, AF.Tanh, f"o_{rnd}_{d}")
                    h = hp.tile([128, 2, SLAB], F32, tag="h", name=f"h_{rnd}_{d}")
                    for ctt in range(2):
                        # h = prev + u * (o - prev)
                        nc.vector.tensor_sub(out=o[:, ctt, :], in0=o[:, ctt, :],
                                             in1=prev[:, ctt, :])
                        nc.vector.tensor_mul(out=o[:, ctt, :], in0=o[:, ctt, :],
                                             in1=u[:, ctt, :])
                        nc.vector.tensor_add(out=h[:, ctt, :],
                                             in0=prev[:, ctt, :],
                                             in1=o[:, ctt, :])
                    if rnd == ROUNDS - 1:
                        h16 = hp.tile([128, 2, SLAB], F16, tag="h16",
                                      name=f"h16_{rnd}_{d}")
                        nc.vector.tensor_copy(out=h16, in_=h)
                        for et in range(2):
                            nc.sync.dma_start(out=out_slab[d, et, :, :],
                                              in_=h16[:, et, :])
                    else:
                        for et in range(2):
                            nc.sync.dma_start(out=DT[rnd]["h_local"][d, et, :, :],
                                              in_=h[:, et, :])
                            nc.sync.dma_start(
                                out=DT[rnd][f"agh_in{d}"][et * 128:et * 128 + 128, :],
                                in_=h[:, et, :])
                        nc.gpsimd.collective_compute(
                            "AllGather", mybir.AluOpType.bypass,
                            replica_groups=[list(range(NCORE))],
                            ins=[DT[rnd][f"agh_in{d}"][:].opt()],
                            outs=[DT[rnd][f"agh_out{d}"][:].opt()])

    nc.compile()
    return nc


# --------------------------- cached PJRT runner ---------------------------
# Same _bass_exec_p -> bass_exec custom-call -> PJRT path that
# bass_utils.run_bass_kernel_spmd takes under axon (see
# concourse.bass2jax.run_bass_via_pjrt), except the jitted shard_map
# executable and the zero output-initializer device buffers are built once
# and reused, so repeat calls skip retracing/recompiling and skip
# re-uploading output buffers. The kernel writes every output element, so
# donation of pre-zeroed outputs is unnecessary.
_RUNNER = None


def _get_runner():
    global _RUNNER
    if _RUNNER is not None:
        return _RUNNER

    import jax
    from jax.sharding import Mesh, PartitionSpec
    from jax.experimental.shard_map import shard_map
    from concourse.bass2jax import (_bass_exec_p, install_neuronx_cc_hook,
                                    partition_id_tensor)

    nc = _build_nc()
    install_neuronx_cc_hook()
    partition_name = (nc.partition_id_tensor.name
                      if nc.partition_id_tensor else None)
    in_names, out_names, out_avals, zero_outs = [], [], [], []
    for alloc in nc.m.functions[0].allocations:
        if not isinstance(alloc, mybir.MemoryLocationSet):
            continue
        name = alloc.memorylocations[0].name
        if alloc.kind == "ExternalInput":
            if name != partition_name:
                in_names.append(name)
        elif alloc.kind == "ExternalOutput":
            out_names.append(name)
            shape = tuple(alloc.tensor_shape)
            dtype = mybir.dt.np(alloc.dtype)
            out_avals.append(jax.core.ShapedArray(shape, dtype))
            zero_outs.append(np.zeros(shape, dtype))
    n_params = len(in_names)
    all_names = in_names + out_names + ([partition_name] if partition_name
                                        else [])

    def _body(*args):
        operands = list(args)
        if partition_name is not None:
            operands.append(partition_id_tensor())
        outs = _bass_exec_p.bind(
            *operands, out_avals=tuple(out_avals), in_names=tuple(all_names),
            out_names=tuple(out_names), lowering_input_output_aliases=(),
            sim_require_finite=True, sim_require_nnan=True, nc=nc)
        return tuple(outs)

    devices = jax.devices()[:NCORE]
    mesh = Mesh(np.asarray(devices), ("core",))
    n_ops = n_params + len(out_names)
    sharded = jax.jit(
        shard_map(_body, mesh=mesh, in_specs=(PartitionSpec("core"),) * n_ops,
                  out_specs=(PartitionSpec("core"),) * len(out_names),
                  check_rep=False),
        keep_unused=True)
    sh = jax.sharding.NamedSharding(mesh, PartitionSpec("core"))
    dev_zeros = [
        jax.device_put(np.zeros((NCORE * z.shape[0], *z.shape[1:]), z.dtype),
                       sh)
        for z in zero_outs
    ]
    assert in_names == ["pack"] and out_names == ["out_slab"], (
        in_names, out_names)
    _RUNNER = (sharded, dev_zeros, [a.shape for a in out_avals])
    return _RUNNER


def _prep_pack(inputs):
    f32 = np.float32
    feats = [np.asarray(inputs[k], f32).reshape(2, 128, D)
             for k in ("infeature1", "infeature2", "infeature3")]

    wpack = np.zeros(W_LEN, f32)
    W_lin = np.asarray(inputs["W_lin"], f32)
    wpack[WOF_LIN:WOF_LIN + 65536] = \
        np.ascontiguousarray(W_lin.T).reshape(-1)
    wg = np.zeros((2, 128, 4), f32)
    wg[:, :, 0] = np.asarray(inputs["W_gate"], f32).reshape(2, 128)
    wpack[WOF_GATE:WOF_GATE + 1024] = wg.reshape(-1)
    wpack[WOF_BCF:WOF_BCF + 256] = np.asarray(inputs["b_cf"], f32)
    wpack[WOF_GRUB:WOF_GRUB + 768] = np.concatenate(
        [np.asarray(inputs[k], f32) for k in ("b_reset", "b_update", "b_out")])
    W_cf = np.asarray(inputs["W_cf"], f32)
    wpack[WOF_CF:WOF_CF + 1179648] = \
        np.ascontiguousarray(W_cf.transpose(2, 3, 1, 0)).reshape(-1)
    wpack[WOF_GRUW:WOF_GRUW + 393216] = np.concatenate(
        [np.ascontiguousarray(np.asarray(inputs[k], f32).T).reshape(-1)
         for k in ("W_reset", "W_update", "W_out")])

    packs = np.zeros((NCORE, PACK_N), f32)
    for r in range(NCORE):
        t0 = r * SLAB
        n = max(0, min(t0 + SLAB, D) - t0)
        fs = packs[r, OF_FEAT:OF_FEAT + LEN_FEAT].reshape(3, 2, 128, SLAB)
        if n > 0:
            for f in range(3):
                fs[f, :, :, :n] = feats[f][:, :, t0:t0 + n]
        packs[r, OF_VALID:OF_VALID + n] = 1.0
        packs[r, OF_HMASK:OF_HMASK + 256].reshape(128, 2)[:, 0] = \
            0.0 if r == 0 else 1.0
        packs[r, OF_HMASK:OF_HMASK + 256].reshape(128, 2)[:, 1] = \
            0.0 if r == NCORE - 1 else 1.0
        packs[r, OF_HB:OF_HB + 4] = np.array(
            [((r + 7) % 8) * 512, ((r + 1) % 8) * 512,
             ((r + 7) % 8) * 1024, ((r + 1) % 8) * 1024],
            np.int32).view(f32)
        packs[r, S_LEN:PACK_N] = wpack[r * WPC:(r + 1) * WPC]
    return packs


def kernel(**inputs):
    sharded, dev_zeros, out_shapes = _get_runner()
    packs = _prep_pack(inputs)
    out_arrs = sharded(packs, *dev_zeros)
    res = np.asarray(out_arrs[0])  # [NCORE*3, 2, 128, SLAB] f16
    res = res.reshape(NCORE, 3, 2, 128, SLAB)
    outs = []
    for f in range(3):
        full = np.zeros((C, D), np.float32)
        for r in range(NCORE):
            t0 = r * SLAB
            n = max(0, min(t0 + SLAB, D) - t0)
            if n > 0:
                sl = res[r, f].reshape(C, SLAB).astype(np.float32)
                full[:, t0:t0 + n] = sl[:, :n]
        outs.append(full.reshape(1, C, HW, HW))
    return tuple(outs)


if __name__ == "__main__":
    # build-only check
    nc = _build_nc()
    print("build OK")


# revision 13
# speedup vs baseline: 16.7282x; 1.3968x over previous
"""Trainium2 Bass kernel for nn_CoattentionModel (co-attention + conv-fusion + convGRU).

Sharding: token axis (3600 tokens = 60x60 image) padded to 64 rows (3840 tokens),
split 8 ways -> each core owns 8 image rows (480 tokens). Attention is computed
as A'[j,i] tiles (query-token j on partitions), softmax without max-subtraction
(logits verified <= ~40), attention output accumulated over 29 j-tiles in PSUM.
Softmax sum + gate row come from a 2-row matmul against [ones | g] per j-tile.
Normalize * sigmoid-gate * pad-valid mask fold into one per-column scale vector.
Matmuls run in float32r (full PE rate, ~1e-3 max rel err); the 3x3 conv path
runs in bf16 to fit SBUF. Per round: 2 edge AllGathers provide conv halos
(read back at rank-dynamic register offsets), 3 feature AllGathers rebuild the
full features for the next round's attention.

Host I/O strategy (the axon relay charges ~50-150ms per transfer + ~60MB/s):
ship ONE f32 array per core [1, PACK_N] holding that core's feature slab plus
1/8th of the replicated weights; the kernel AllGathers the weight shards and
the feature slabs on device. Output returns as f16 (halves download bytes).
The jitted shard_map executable is built once and cached — per-call cost is
upload + execute + download only. This is the same _bass_exec_p/PJRT path
run_bass_kernel_spmd takes under axon, minus its per-call retrace.
"""
import sys
for _p in ("/opt/trn_rl_repo", "/root/.axon_site/_ro/trn_rl_repo"):
    if _p not in sys.path:
        sys.path.insert(0, _p)

import numpy as np

import concourse.bass as bass
import concourse.mybir as mybir
import concourse.tile as tile
from concourse import bacc
from concourse.masks import make_identity

F32 = mybir.dt.float32
F32R = mybir.dt.float32r
F16 = mybir.dt.float16
BF16 = mybir.dt.bfloat16
I32 = mybir.dt.int32
AF = mybir.ActivationFunctionType
MUL = mybir.AluOpType.mult

C = 256
HW = 60
D = HW * HW              # 3600
ROWS_PAD = 64
D_PAD = ROWS_PAD * HW    # 3840
NCORE = 8
SLAB = D_PAD // NCORE    # 480
PW = HW + 2              # padded image width
ROUNDS = 5
JT = [(s, min(s + 128, D)) for s in range(0, D, 128)]  # 29 j-tiles over REAL tokens
NJT = len(JT)

# attention list: (E feature, Q feature), grouped in pairs sharing Q
ATTS = [(0, 1), (2, 1), (0, 2), (1, 2), (1, 0), (2, 0)]
PAIRS = [(1, [0, 1]), (2, [2, 3]), (0, [4, 5])]  # (Q feature, att indices)
# conv d consumes (attA, attB) channel-concat; GRU prev = feature d
CONV_PARTS = [(0, 2), (4, 3), (5, 1)]
# edge AllGather membership: AG-a = atts {0, 2} (ready after pair2) -> conv1
#                            AG-b = atts {1, 3, 4, 5} -> conv2, conv3
AG_A_ATTS = [0, 2]
AG_B_ATTS = [1, 3, 4, 5]

# ---------------- packed-input layout (f16 element offsets) ----------------
# The pack is ONE f16 array per core: [feature slab | weight shard] is a
# contiguous 1121x512 block that a single AllGather replicates; the trailing
# 3x512 carries per-core masks/halo bases (halo bases stored as exactly-
# representable f16 values, converted to int32 on device).
OF_FEAT = 0                      # [3, 2, 128, SLAB] own feature slab
LEN_FEAT = 3 * 2 * 128 * SLAB    # 368640  (= 720 * 512)
OF_WSH = 368640                  # this core's 1/8 of the weight pack
W_ROWS_PC = 401                  # weight-shard rows ([512-wide]) per core
WPC = W_ROWS_PC * 512            # 205312
GAG_ROWS = 720 + W_ROWS_PC       # 1121 rows gathered per core
OF_VALID = 573952                # [1, SLAB] valid-token mask
OF_HMASK = 574464                # [128, 2] halo edge mask
OF_HB = 574976                   # [1, 4] halo base rows (as f16 values)
PACK_N = 575488
# Weight-pack layout (flat offsets into the 8-shard concatenation):
WOF_LIN = 0                      # [2, 128, 256] W_lin^T
WOF_GATE = 65536                 # [2, 128, 4]   gate weight (col 0)
WOF_BCF = 66560                  # [2, 128]      conv bias
WOF_GRUB = 67072                 # [3, 2, 128]   GRU biases
WOF_CF = 68096                   # [9, 4, 128, 256] conv_fusion taps
WOF_GRUW = 1247744               # [3, 4, 128, 256] GRU weights
W_LEN = 1642496                  # 401 * 4096 (padded to 8*512 multiple)


def r32(ap):
    return ap.bitcast(F32R)


def _build_nc():
    nc = bacc.Bacc("TRN2", target_bir_lowering=False, debug=False,
                   num_devices=NCORE)

    pack = nc.dram_tensor("pack", [1, PACK_N], F16, kind="ExternalInput")
    out_slab = nc.dram_tensor("out_slab", [3, 2, 128, SLAB], F16,
                              kind="ExternalOutput")

    with tile.TileContext(nc) as tc:
        import contextlib
        ctx = contextlib.ExitStack()
        with ctx:
            cst = ctx.enter_context(tc.tile_pool(name="cst", bufs=1))
            qfp = ctx.enter_context(tc.tile_pool(name="qfp", bufs=1))
            qtp = ctx.enter_context(tc.tile_pool(name="qtp", bufs=1))
            sgp = ctx.enter_context(tc.tile_pool(name="sgp", bufs=1))
            eslp = ctx.enter_context(tc.tile_pool(name="eslp", bufs=2))
            crp = ctx.enter_context(tc.tile_pool(name="crp", bufs=2))
            epp = ctx.enter_context(tc.tile_pool(name="epp", bufs=4))
            attp = ctx.enter_context(tc.tile_pool(name="attp", bufs=8))
            vecp = ctx.enter_context(tc.tile_pool(name="vecp", bufs=6))
            scbp = ctx.enter_context(tc.tile_pool(name="scbp", bufs=2))
            padp = ctx.enter_context(tc.tile_pool(name="padp", bufs=1))
            asbp = ctx.enter_context(tc.tile_pool(name="asbp", bufs=2))
            prvp = ctx.enter_context(tc.tile_pool(name="prvp", bufs=2))
            grup = ctx.enter_context(tc.tile_pool(name="grup", bufs=3))
            hp = ctx.enter_context(tc.tile_pool(name="hp", bufs=2))
            ps = ctx.enter_context(tc.tile_pool(name="ps", bufs=1, space="PSUM"))
            dr = ctx.enter_context(tc.tile_pool(name="dr", bufs=1, space="DRAM"))

            # ---- rebuild replicated weights + full features on device ----
            # One AllGather replicates each core's [feature slab | weight
            # shard] block; the weight shards are then compacted into a
            # contiguous DRAM pack so flat views can address each piece.
            gag_in = dr.tile([GAG_ROWS, 512], F16, name="gag_in")
            nc.sync.dma_start(
                out=gag_in,
                in_=pack[0, 0:GAG_ROWS * 512].rearrange("(r c) -> r c", c=512))
            gag_out = dr.tile([GAG_ROWS * NCORE, 512], F16,
                              addr_space="Shared", name="gag_out")
            nc.gpsimd.collective_compute(
                "AllGather", mybir.AluOpType.bypass,
                replica_groups=[list(range(NCORE))],
                ins=[gag_in[:].opt()], outs=[gag_out[:].opt()])

            wfull = dr.tile([W_ROWS_PC * NCORE, 512], F16, name="wfull")
            for r in range(NCORE):
                nc.sync.dma_start(
                    out=wfull[r * W_ROWS_PC:(r + 1) * W_ROWS_PC, :],
                    in_=gag_out[r * GAG_ROWS + 720:
                                r * GAG_ROWS + GAG_ROWS, :])

            wflat = wfull[:].rearrange("r c -> (r c)")

            def wsl(ofs, n):
                return wflat[ofs:ofs + n]

            wstg = ctx.enter_context(tc.tile_pool(name="wstg", bufs=2))
            wst2 = ctx.enter_context(tc.tile_pool(name="wst2", bufs=1))

            def load_w(shape, ofs, n, pat, dtype, **axes):
                # one-shot startup staging; per-piece tag (sizes differ)
                stg = wst2.tile(shape, F16, tag=f"stg{ofs}",
                                name=f"wstg_{ofs}")
                nc.sync.dma_start(out=stg, in_=wsl(ofs, n).rearrange(pat, **axes))
                dst = cst.tile(shape, dtype, tag=f"w{ofs}", name=f"w_{ofs}")
                nc.vector.tensor_copy(out=dst, in_=stg)
                return dst

            # ------------- constants -------------
            wlin_sb = load_w([128, 2, 256], WOF_LIN, 65536,
                             "(k p e) -> p k e", F32R, k=2, p=128)
            wgate_sb = load_w([128, 2, 4], WOF_GATE, 1024,
                              "(k p n) -> p k n", F32R, k=2, p=128)
            bcf_sb = load_w([128, 2], WOF_BCF, 256,
                            "(c p) -> p c", F32, c=2)
            grub_sb = load_w([128, 3, 2], WOF_GRUB, 768,
                             "(g c p) -> p g c", F32, g=3, c=2)
            gruw_sb = load_w([128, 3, 4, 256], WOF_GRUW, 393216,
                             "(g k p o) -> p g k o", F32R, g=3, k=4, p=128)
            wcf_sb = cst.tile([128, 9, 4, 256], BF16)
            for t in range(9):
                stg = wstg.tile([128, 4, 256], F16, tag="stg",
                                name=f"wcfstg_{t}")
                nc.sync.dma_start(
                    out=stg,
                    in_=wsl(WOF_CF + t * 131072, 131072).rearrange(
                        "(k p o) -> p k o", k=4, p=128))
                nc.vector.tensor_copy(out=wcf_sb[:, t, :, :], in_=stg)
            hmask16 = cst.tile([128, 2], F16)
            nc.sync.dma_start(
                out=hmask16,
                in_=pack[0, OF_HMASK:OF_HMASK + 256].rearrange(
                    "(p c) -> p c", c=2))
            hmask_sb = cst.tile([128, 2], F32)
            nc.vector.tensor_copy(out=hmask_sb, in_=hmask16)
            valid16 = cst.tile([1, SLAB], F16)
            nc.sync.dma_start(
                out=valid16,
                in_=pack[0, OF_VALID:OF_VALID + SLAB].rearrange(
                    "(a s) -> a s", a=1))
            valid_sb = cst.tile([1, SLAB], F32)
            nc.vector.tensor_copy(out=valid_sb, in_=valid16)
            ident_f = cst.tile([128, 128], F32)
            make_identity(nc, ident_f)
            ident = cst.tile([128, 128], F32R)
            nc.vector.tensor_copy(out=ident, in_=ident_f)
            ones_f = cst.tile([128, NJT], F32)
            nc.vector.memset(ones_f, 1.0)

            # halo base registers (Pool engine, persistent).  The bases ride
            # in the f16 pack as exact values; convert f16 -> f32 -> int32.
            hb16 = cst.tile([1, 4], F16)
            nc.sync.dma_start(
                out=hb16,
                in_=pack[0, OF_HB:OF_HB + 4].rearrange("(a b) -> a b", a=1))
            hbf = cst.tile([1, 4], F32)
            nc.vector.tensor_copy(out=hbf, in_=hb16)
            hb_sb = cst.tile([1, 4], I32)
            nc.vector.tensor_copy(out=hb_sb, in_=hbf)
            halo_vals = []
            for i in range(4):
                reg = nc.alloc_registers(f"halo_reg{i}",
                                         engines=[mybir.EngineType.Pool])
                nc.reg_load(list(reg), hb_sb[0:1, i:i + 1])
                halo_vals.append(nc.snap(reg, donate=False))

            def own_feat(f, et):
                ofs = OF_FEAT + (f * 2 + et) * 128 * SLAB
                return pack[0, ofs:ofs + 128 * SLAB].rearrange(
                    "(p s) -> p s", s=SLAB)

            gflat = gag_out[:].rearrange("r c -> (r c)")

            def core_feat(b, f, et):
                # core b's [128, SLAB] slab of feature f half et in gag_out
                ofs = b * GAG_ROWS * 512 + (f * 2 + et) * 128 * SLAB
                return gflat[ofs:ofs + 128 * SLAB].rearrange(
                    "(p s) -> p s", s=SLAB)

            f16p = ctx.enter_context(tc.tile_pool(name="f16p", bufs=2))

            # per-round DRAM buffers
            def dram_tiles():
                out = []
                for rnd in range(ROUNDS):
                    t = {}
                    t["aga_in"] = dr.tile([512, 120], BF16, tag="aga_in", bufs=2,
                                          name=f"aga_in_{rnd}")
                    t["aga_out"] = dr.tile([512 * NCORE, 120], BF16,
                                           addr_space="Shared", tag="aga_out",
                                           bufs=2, name=f"aga_out_{rnd}")
                    t["agb_in"] = dr.tile([1024, 120], BF16, tag="agb_in", bufs=2,
                                          name=f"agb_in_{rnd}")
                    t["agb_out"] = dr.tile([1024 * NCORE, 120], BF16,
                                           addr_space="Shared", tag="agb_out",
                                           bufs=2, name=f"agb_out_{rnd}")
                    t["h_local"] = dr.tile([3, 2, 128, SLAB], F32, tag="h_local",
                                           bufs=2, name=f"h_local_{rnd}")
                    if rnd < ROUNDS - 1:
                        for f in range(3):
                            t[f"agh_in{f}"] = dr.tile(
                                [256, SLAB], F32, tag=f"agh_in{f}", bufs=2,
                                name=f"agh_in{f}_{rnd}")
                            t[f"agh_out{f}"] = dr.tile(
                                [256 * NCORE, SLAB], F32, addr_space="Shared",
                                tag=f"agh_out{f}", bufs=2,
                                name=f"agh_out{f}_{rnd}")
                    out.append(t)
                return out

            DT = dram_tiles()

            for rnd in range(ROUNDS):
                att_bf = {}   # att idx -> bf16 [128, 2, SLAB] tile

                for (qf, att_ids) in PAIRS:
                    # ---------- pre-phase: load Q, build QT + g ----------
                    qfull = qfp.tile([128, 2, D], F32R, tag="qfull",
                                     name=f"qfull_{rnd}_{qf}")
                    for b in range(NCORE):
                        lo = b * SLAB
                        hi = min(lo + SLAB, D)
                        if hi <= lo:
                            continue
                        for et in range(2):
                            if rnd == 0:
                                stg = f16p.tile([128, SLAB], F16, tag="qstg",
                                                name=f"qstg_{qf}_{b}_{et}")
                                nc.sync.dma_start(
                                    out=stg[:, 0:hi - lo],
                                    in_=core_feat(b, qf, et)[:, 0:hi - lo])
                                nc.vector.tensor_copy(
                                    out=qfull[:, et, lo:hi],
                                    in_=stg[:, 0:hi - lo])
                            else:
                                src = DT[rnd - 1][f"agh_out{qf}"]
                                row = b * 256 + et * 128
                                nc.sync.dma_start(
                                    out=qfull[:, et, lo:hi],
                                    in_=src[row:row + 128,
                                            0:hi - lo].bitcast(F32R))

                    qt = qtp.tile([128, NJT, 256], F32R, tag="qt",
                                  name=f"qt_{rnd}_{qf}")
                    sg = sgp.tile([128, NJT, 2], F32R, tag="sg",
                                  name=f"sg_{rnd}_{qf}")
                    nc.vector.tensor_copy(out=sg[:, :, 0], in_=ones_f)
                    for jt, (js, je) in enumerate(JT):
                        jsz = je - js
                        for et in range(2):
                            tp = ps.tile([128, 128], F32R, tag="big",
                                         bufs=3, name=f"tp_{rnd}_{qf}_{jt}_{et}")
                            nc.tensor.matmul(tp[:jsz, :],
                                             qfull[:, et, js:je],
                                             ident[:], is_transpose=True,
                                             start=True, stop=True)
                            nc.any.tensor_copy(
                                out=qt[:jsz, jt, et * 128:(et + 1) * 128],
                                in_=tp[:jsz, :])
                        gp = ps.tile([128, 4], F32, tag="big", bufs=3,
                                     name=f"gp_{rnd}_{qf}_{jt}")
                        for kt in range(2):
                            nc.tensor.matmul(gp[:jsz, :],
                                             qfull[:, kt, js:je],
                                             wgate_sb[:, kt, :],
                                             start=(kt == 0), stop=(kt == 1))
                        nc.any.tensor_copy(out=sg[:jsz, jt, 1:2], in_=gp[:jsz, 0:1])

                    # ---------- corr_T for both atts ----------
                    corrs = []
                    for ai in att_ids:
                        e = ATTS[ai][0]
                        esl = eslp.tile([128, 2, SLAB], F32R, tag="esl",
                                        name=f"esl_{rnd}_{ai}")
                        for et in range(2):
                            if rnd == 0:
                                stg = f16p.tile([128, SLAB], F16, tag="qstg",
                                                name=f"estg_{ai}_{et}")
                                nc.sync.dma_start(out=stg, in_=own_feat(e, et))
                                nc.vector.tensor_copy(out=esl[:, et, :], in_=stg)
                            else:
                                nc.sync.dma_start(
                                    out=esl[:, et, :],
                                    in_=DT[rnd - 1]["h_local"][e, et, :, :].bitcast(F32R))
                        csb = crp.tile([128, 2, SLAB], F32R, tag="corrT",
                                       name=f"csb_{rnd}_{ai}")
                        for eo in range(2):
                            pc = ps.tile([128, SLAB], F32, tag="big", bufs=3,
                                         name=f"pc_{rnd}_{ai}_{eo}")
                            for kt in range(2):
                                nc.tensor.matmul(
                                    pc, wlin_sb[:, kt, eo * 128:(eo + 1) * 128],
                                    esl[:, kt, :],
                                    start=(kt == 0), stop=(kt == 1))
                            nc.any.tensor_copy(out=csb[:, eo, :], in_=pc)
                        corrs.append(csb)

                    # ---------- j-loop ----------
                    att_ps = []
                    sums_acc = []
                    for k, ai in enumerate(att_ids):
                        for ctt in range(2):
                            att_ps.append(ps.tile(
                                [128, SLAB], F32, tag="acc", bufs=4,
                                name=f"attps_{rnd}_{ai}_{ctt}"))
                        sums_acc.append(vecp.tile(
                            [2, SLAB], F32, tag="vec", name=f"sums_{rnd}_{ai}"))
                    for jt, (js, je) in enumerate(JT):
                        jsz = je - js
                        for k, ai in enumerate(att_ids):
                            ap = ps.tile([128, SLAB], F32, tag="big", bufs=3,
                                         name=f"ap_{rnd}_{ai}_{jt}")
                            for kt in range(2):
                                nc.tensor.matmul(ap[:jsz, :],
                                                 qfull[:, kt, js:je],
                                                 corrs[k][:, kt, :],
                                                 start=(kt == 0), stop=(kt == 1))
                            eb = epp.tile([128, SLAB], F32R, tag="ep",
                                          name=f"eb_{rnd}_{ai}_{jt}")
                            nc.scalar.activation(eb[:jsz, :], ap[:jsz, :], AF.Exp)
                            sp = ps.tile([2, SLAB], F32, tag="big", bufs=3,
                                         name=f"sp_{rnd}_{ai}_{jt}")
                            nc.tensor.matmul(sp, sg[:jsz, jt, :],
                                             eb[:jsz, :],
                                             start=True, stop=True)
                            if jt == 0:
                                nc.vector.tensor_copy(out=sums_acc[k], in_=sp)
                            else:
                                nc.vector.tensor_add(out=sums_acc[k],
                                                     in0=sums_acc[k], in1=sp)
                            for ctt in range(2):
                                nc.tensor.matmul(
                                    att_ps[k * 2 + ctt],
                                    qt[:jsz, jt, ctt * 128:(ctt + 1) * 128],
                                    eb[:jsz, :],
                                    start=(jt == 0), stop=(jt == NJT - 1))

                    # ---------- epilogue per att ----------
                    for k, ai in enumerate(att_ids):
                        recip = vecp.tile([2, SLAB], F32, tag="vec",
                                          name=f"recip_{rnd}_{ai}")
                        nc.vector.reciprocal(recip[0:1, :], sums_acc[k][0:1, :])
                        gr0 = vecp.tile([2, SLAB], F32, tag="vec",
                                        name=f"gr0_{rnd}_{ai}")
                        nc.sync.dma_start(out=gr0[0:1, :],
                                          in_=sums_acc[k][1:2, :])
                        scv = vecp.tile([2, SLAB], F32, tag="vec",
                                        name=f"scv_{rnd}_{ai}")
                        nc.vector.tensor_mul(out=scv[0:1, :], in0=gr0[0:1, :],
                                             in1=recip[0:1, :])
                        nc.scalar.activation(scv[0:1, :], scv[0:1, :], AF.Sigmoid)
                        nc.vector.tensor_mul(out=scv[0:1, :], in0=scv[0:1, :],
                                             in1=recip[0:1, :])
                        nc.vector.tensor_mul(out=scv[0:1, :], in0=scv[0:1, :],
                                             in1=valid_sb[0:1, :])
                        scd = dr.tile([1, SLAB], F32, tag="scvd", bufs=2,
                                      name=f"scd_{rnd}_{ai}")
                        nc.sync.dma_start(out=scd, in_=scv[0:1, :])
                        scb = scbp.tile([128, SLAB], F32, tag="scb",
                                        name=f"scb_{rnd}_{ai}")
                        nc.sync.dma_start(out=scb,
                                          in_=scd[0:1, :].partition_broadcast(128))
                        abf = attp.tile([128, 2, SLAB], BF16, tag="attbf",
                                        name=f"abf_{rnd}_{ai}")
                        for ctt in range(2):
                            nc.vector.tensor_tensor(out=abf[:, ctt, :],
                                                    in0=att_ps[k * 2 + ctt],
                                                    in1=scb, op=MUL)
                        att_bf[ai] = abf
                        # edge writes into the AG bounce this att belongs to
                        if ai in AG_A_ATTS:
                            bounce, loc = DT[rnd]["aga_in"], AG_A_ATTS.index(ai)
                        else:
                            bounce, loc = DT[rnd]["agb_in"], AG_B_ATTS.index(ai)
                        for et in range(2):
                            row = loc * 256 + et * 128
                            nc.sync.dma_start(out=bounce[row:row + 128, 0:60],
                                              in_=abf[:, et, 0:60])
                            nc.sync.dma_start(out=bounce[row:row + 128, 60:120],
                                              in_=abf[:, et, SLAB - 60:SLAB])

                    # fire edge collectives at pair boundaries
                    if qf == 2:  # after pair2 (atts 0..3 done; AG-a atts ready)
                        nc.gpsimd.collective_compute(
                            "AllGather", mybir.AluOpType.bypass,
                            replica_groups=[list(range(NCORE))],
                            ins=[DT[rnd]["aga_in"][:].opt()],
                            outs=[DT[rnd]["aga_out"][:].opt()])
                    if qf == 0:  # after pair3
                        nc.gpsimd.collective_compute(
                            "AllGather", mybir.AluOpType.bypass,
                            replica_groups=[list(range(NCORE))],
                            ins=[DT[rnd]["agb_in"][:].opt()],
                            outs=[DT[rnd]["agb_out"][:].opt()])

                # ---------- convs + GRUs ----------
                for d in range(3):
                    pa, pb = CONV_PARTS[d]
                    inp = padp.tile([128, 4, 622], BF16, tag="inpad",
                                    name=f"inp_{rnd}_{d}")
                    nc.vector.memset(inp, 0.0)
                    for part, ai in enumerate((pa, pb)):
                        for et in range(2):
                            kt = part * 2 + et
                            # own tokens at cols 64 + 62*row
                            dst = inp[:, kt, 64:64 + 8 * PW].rearrange(
                                "p (r w) -> p r w", w=PW)[:, :, 0:HW]
                            src = att_bf[ai][:, et, :].rearrange(
                                "p (r w) -> p r w", w=HW)
                            nc.sync.dma_start(out=dst, in_=src)
                            # halos
                            if ai in AG_A_ATTS:
                                agout = DT[rnd]["aga_out"]
                                loc = AG_A_ATTS.index(ai)
                                lval, rval = halo_vals[0], halo_vals[1]
                            else:
                                agout = DT[rnd]["agb_out"]
                                loc = AG_B_ATTS.index(ai)
                                lval, rval = halo_vals[2], halo_vals[3]
                            row = loc * 256 + et * 128
                            nc.gpsimd.dma_start(
                                out=inp[:, kt, 2:62],
                                in_=agout[row:][bass.ds(lval, 128), 60:120])
                            nc.vector.tensor_scalar_mul(
                                out=inp[:, kt, 2:62], in0=inp[:, kt, 2:62],
                                scalar1=hmask_sb[:, 0:1])
                            nc.gpsimd.dma_start(
                                out=inp[:, kt, 560:620],
                                in_=agout[row:][bass.ds(rval, 128), 0:60])
                            nc.vector.tensor_scalar_mul(
                                out=inp[:, kt, 560:620], in0=inp[:, kt, 560:620],
                                scalar1=hmask_sb[:, 1:2])

                    a_sb = asbp.tile([128, 2, SLAB], F32R, tag="asb",
                                     name=f"asb_{rnd}_{d}")
                    for ctt in range(2):
                        cp = ps.tile([128, 497], F32, tag="conv", bufs=1,
                                     name=f"cp_{rnd}_{d}_{ctt}")
                        first = True
                        for kt in range(4):
                            for ky in range(3):
                                for kx in range(3):
                                    dpp = (ky - 1) * PW + (kx - 1)
                                    nc.tensor.matmul(
                                        cp[:, 0:496],
                                        wcf_sb[:, ky * 3 + kx, kt,
                                               ctt * 128:(ctt + 1) * 128],
                                        inp[:, kt, 63 + dpp:63 + dpp + 496],
                                        start=first,
                                        stop=(kt == 3 and ky == 2 and kx == 2))
                                    first = False
                        cpx = cp[:, 1:1 + 8 * PW].rearrange(
                            "p (r w) -> p r w", w=PW)[:, :, 0:HW]
                        nc.vector.tensor_scalar_add(
                            out=a_sb[:, ctt, :].rearrange("p (r w) -> p r w", w=HW),
                            in0=cpx, scalar1=bcf_sb[:, ctt:ctt + 1])

                    # ---- GRU d ----
                    prev = prvp.tile([128, 2, SLAB], F32R, tag="prev",
                                     name=f"prev_{rnd}_{d}")
                    for et in range(2):
                        if rnd == 0:
                            stg = f16p.tile([128, SLAB], F16, tag="qstg",
                                            name=f"pstg_{d}_{et}")
                            nc.sync.dma_start(out=stg, in_=own_feat(d, et))
                            nc.vector.tensor_copy(out=prev[:, et, :], in_=stg)
                        else:
                            nc.sync.dma_start(
                                out=prev[:, et, :],
                                in_=DT[rnd - 1]["h_local"][d, et, :, :].bitcast(F32R))

                    def gate1x1(gate_i, rhs_pairs, func, outname):
                        gt = grup.tile([128, 2, SLAB], F32, tag="grutmp",
                                       name=outname)
                        for ctt in range(2):
                            gps = ps.tile([128, SLAB], F32, tag="conv", bufs=1,
                                          name=f"{outname}_ps{ctt}")
                            for kt in range(4):
                                nc.tensor.matmul(
                                    gps,
                                    gruw_sb[:, gate_i, kt,
                                                ctt * 128:(ctt + 1) * 128],
                                    rhs_pairs[kt],
                                    start=(kt == 0), stop=(kt == 3))
                            nc.scalar.activation(
                                gt[:, ctt, :], gps, func,
                                bias=grub_sb[:, gate_i, ctt:ctt + 1])
                        return gt

                    st = [a_sb[:, 0, :], a_sb[:, 1, :], prev[:, 0, :],
                          prev[:, 1, :]]
                    # gru_W order: 0=reset, 1=update, 2=out
                    u = gate1x1(1, st, AF.Sigmoid, f"u_{rnd}_{d}")
                    rg = gate1x1(0, st, AF.Sigmoid, f"r_{rnd}_{d}")
                    pr = grup.tile([128, 2, SLAB], F32R, tag="grutmp",
                                   name=f"pr_{rnd}_{d}")
                    for ctt in range(2):
                        nc.vector.tensor_mul(out=pr[:, ctt, :],
                                             in0=prev[:, ctt, :],
                                             in1=rg[:, ctt, :])
                    st2 = [a_sb[:, 0, :], a_sb[:, 1, :], pr[:, 0, :], pr[:, 1, :]]
                    o = gate1x1(2, st2, AF.Tanh, f"o_{rnd}_{d}")
                    h = hp.tile([128, 2, SLAB], F32, tag="h", name=f"h_{rnd}_{d}")
                    for ctt in range(2):
                        # h = prev + u * (o - prev)
                        nc.vector.tensor_sub(out=o[:, ctt, :], in0=o[:, ctt, :],
                                             in1=prev[:, ctt, :])
                        nc.vector.tensor_mul(out=o[:, ctt, :], in0=o[:, ctt, :],
                                             in1=u[:, ctt, :])
                        nc.vector.tensor_add(out=h[:, ctt, :],
                                             in0=prev[:, ctt, :],
                                             in1=o[:, ctt, :])
                    if rnd == ROUNDS - 1:
                        h16 = hp.tile([128, 2, SLAB], F16, tag="h16",
                                      name=f"h16_{rnd}_{d}")
                        nc.vector.tensor_copy(out=h16, in_=h)
                        for et in range(2):
                            nc.sync.dma_start(out=out_slab[d, et, :, :],
                                              in_=h16[:, et, :])
                    else:
                        for et in range(2):
                            nc.sync.dma_start(out=DT[rnd]["h_local"][d, et, :, :],
                                              in_=h[:, et, :])
                            nc.sync.dma_start(
                                out=DT[rnd][f"agh_in{d}"][et * 128:et * 128 + 128, :],
                                in_=h[:, et, :])
                        nc.gpsimd.collective_compute(
                            "AllGather", mybir.AluOpType.bypass,
                            replica_groups=[list(range(NCORE))],
                            ins=[DT[rnd][f"agh_in{d}"][:].opt()],
                            outs=[DT[rnd][f"agh_out{d}"][:].opt()])

    nc.compile()
    return nc


# --------------------------- cached PJRT runner ---------------------------
# Same _bass_exec_p -> bass_exec custom-call -> PJRT path that
# bass_utils.run_bass_kernel_spmd takes under axon (see
# concourse.bass2jax.run_bass_via_pjrt), except the jitted shard_map
# executable and the zero output-initializer device buffers are built once
# and reused, so repeat calls skip retracing/recompiling and skip
# re-uploading output buffers. The kernel writes every output element, so
# donation of pre-zeroed outputs is unnecessary.
_RUNNER = None


def _get_runner():
    global _RUNNER
    if _RUNNER is not None:
        return _RUNNER

    import jax
    from jax.sharding import Mesh, PartitionSpec
    from jax.experimental.shard_map import shard_map
    from concourse.bass2jax import (_bass_exec_p, install_neuronx_cc_hook,
                                    partition_id_tensor)

    nc = _build_nc()
    install_neuronx_cc_hook()
    partition_name = (nc.partition_id_tensor.name
                      if nc.partition_id_tensor else None)
    in_names, out_names, out_avals, zero_outs = [], [], [], []
    for alloc in nc.m.functions[0].allocations:
        if not isinstance(alloc, mybir.MemoryLocationSet):
            continue
        name = alloc.memorylocations[0].name
        if alloc.kind == "ExternalInput":
            if name != partition_name:
                in_names.append(name)
        elif alloc.kind == "ExternalOutput":
            out_names.append(name)
            shape = tuple(alloc.tensor_shape)
            dtype = mybir.dt.np(alloc.dtype)
            out_avals.append(jax.core.ShapedArray(shape, dtype))
            zero_outs.append(np.zeros(shape, dtype))
    n_params = len(in_names)
    all_names = in_names + out_names + ([partition_name] if partition_name
                                        else [])

    def _body(*args):
        operands = list(args)
        if partition_name is not None:
            operands.append(partition_id_tensor())
        outs = _bass_exec_p.bind(
            *operands, out_avals=tuple(out_avals), in_names=tuple(all_names),
            out_names=tuple(out_names), lowering_input_output_aliases=(),
            sim_require_finite=True, sim_require_nnan=True, nc=nc)
        return tuple(outs)

    devices = jax.devices()[:NCORE]
    mesh = Mesh(np.asarray(devices), ("core",))
    n_ops = n_params + len(out_names)
    sharded = jax.jit(
        shard_map(_body, mesh=mesh, in_specs=(PartitionSpec("core"),) * n_ops,
                  out_specs=(PartitionSpec("core"),) * len(out_names),
                  check_rep=False),
        keep_unused=True)
    sh = jax.sharding.NamedSharding(mesh, PartitionSpec("core"))
    dev_zeros = [
        jax.device_put(np.zeros((NCORE * z.shape[0], *z.shape[1:]), z.dtype),
                       sh)
        for z in zero_outs
    ]
    assert in_names == ["pack"] and out_names == ["out_slab"], (
        in_names, out_names)
    _RUNNER = (sharded, dev_zeros, [a.shape for a in out_avals])
    return _RUNNER


def _prep_pack(inputs):
    f16 = np.float16
    f32 = np.float32
    feats = [np.asarray(inputs[k], f32).reshape(2, 128, D).astype(f16)
             for k in ("infeature1", "infeature2", "infeature3")]

    wpack = np.zeros(W_LEN, f16)
    W_lin = np.asarray(inputs["W_lin"], f32)
    wpack[WOF_LIN:WOF_LIN + 65536] = \
        np.ascontiguousarray(W_lin.T).reshape(-1)
    wg = np.zeros((2, 128, 4), f32)
    wg[:, :, 0] = np.asarray(inputs["W_gate"], f32).reshape(2, 128)
    wpack[WOF_GATE:WOF_GATE + 1024] = wg.reshape(-1)
    wpack[WOF_BCF:WOF_BCF + 256] = np.asarray(inputs["b_cf"], f32)
    wpack[WOF_GRUB:WOF_GRUB + 768] = np.concatenate(
        [np.asarray(inputs[k], f32) for k in ("b_reset", "b_update", "b_out")])
    W_cf = np.asarray(inputs["W_cf"], f32)
    wpack[WOF_CF:WOF_CF + 1179648] = \
        np.ascontiguousarray(W_cf.transpose(2, 3, 1, 0)).reshape(-1)
    wpack[WOF_GRUW:WOF_GRUW + 393216] = np.concatenate(
        [np.ascontiguousarray(np.asarray(inputs[k], f32).T).reshape(-1)
         for k in ("W_reset", "W_update", "W_out")])

    packs = np.zeros((NCORE, PACK_N), f16)
    for r in range(NCORE):
        t0 = r * SLAB
        n = max(0, min(t0 + SLAB, D) - t0)
        fs = packs[r, OF_FEAT:OF_FEAT + LEN_FEAT].reshape(3, 2, 128, SLAB)
        if n > 0:
            for f in range(3):
                fs[f, :, :, :n] = feats[f][:, :, t0:t0 + n]
        packs[r, OF_WSH:OF_WSH + WPC] = wpack[r * WPC:(r + 1) * WPC]
        packs[r, OF_VALID:OF_VALID + n] = 1.0
        packs[r, OF_HMASK:OF_HMASK + 256].reshape(128, 2)[:, 0] = \
            0.0 if r == 0 else 1.0
        packs[r, OF_HMASK:OF_HMASK + 256].reshape(128, 2)[:, 1] = \
            0.0 if r == NCORE - 1 else 1.0
        # halo bases: multiples of 512 <= 7168, all exactly representable
        packs[r, OF_HB:OF_HB + 4] = np.array(
            [((r + 7) % 8) * 512, ((r + 1) % 8) * 512,
             ((r + 7) % 8) * 1024, ((r + 1) % 8) * 1024], f16)
    return packs


def kernel(**inputs):
    sharded, dev_zeros, out_shapes = _get_runner()
    packs = _prep_pack(inputs)
    out_arrs = sharded(packs, *dev_zeros)
    res = np.asarray(out_arrs[0])  # [NCORE*3, 2, 128, SLAB] f16
    res = res.reshape(NCORE, 3, 2, 128, SLAB)
    outs = []
    for f in range(3):
        full = np.zeros((C, D), np.float32)
        for r in range(NCORE):
            t0 = r * SLAB
            n = max(0, min(t0 + SLAB, D) - t0)
            if n > 0:
                sl = res[r, f].reshape(C, SLAB).astype(np.float32)
                full[:, t0:t0 + n] = sl[:, :n]
        outs.append(full.reshape(1, C, HW, HW))
    return tuple(outs)


if __name__ == "__main__":
    # build-only check
    nc = _build_nc()
    print("build OK")
